# revision 1
# baseline (speedup 1.0000x reference)
"""Trainium2 Bass kernel for batched multi-head self-attention.

Reference computation (per batch element b):
    qkv = x @ w_qkv.T                  # [N, 3C]
    q, k, v = split/reshape to heads   # H=16 heads, d=64
    attn = softmax(q @ k.T / sqrt(d))
    out = (attn @ v) reshaped back     # [N, C]
    y = out @ w_proj.T + b_proj

Sharding: pure data-parallel over batch B=8 across the 8 NeuronCores
(one batch element per core, weights replicated, no collectives).

On-device layout (everything transposed so matmuls contract over the
partition axis with no on-device transposes):
  - xT      [C, N]   (host pre-transposed, bf16)
  - wqkvT   [C, 3C]  (host pre-transposed, bf16)
  - wprojT  [C, C]   (host pre-transposed, bf16)
  - scores computed as S^T tiles [m, n]; softmax row-sums obtained by
    appending a ones-column to V in the attn@V matmul (PE computes the
    sums for free); normalization applied at PSUM evacuation.

Performance structure (TimelineSim: 229.2us, PE 96.5% busy; the
all-matmul floor is 218.4us = 1024 MMs x 213ns):
  - bf16 matmuls everywhere (fp32 is 4x slower on the PE); fp32 PSUM
    accumulation and fp32 softmax scores keep rel err ~6e-3.
  - QK^T head pairs are row-packed via tile_position (K=64 each, rows
    0-63 / 64-127) -- concurrent on silicon.
  - Emission interleaves attention (ACT-heavy) with the q/k projection
    (PE-heavy) per head-pair so the scheduler fills softmax-bound PE
    bubbles with projection matmuls; output projection is emitted
    n2-outer to fill the last pair's tail.
  - PSUM budget (8 banks): acc 3 + st 3 + av 2; attention loops
    n2-outer so only one n2's AV accumulators are live. Phases whose
    natural tags are idle borrow the other tags' slots (v-projection
    rotates across all three; the final projection chains spread over
    all 8 slots so their preludes run before the last pair lands).
  - Dummy warm-up matmuls during the initial DMA wait complete the
    PE p-state/HAM ramp before real work arrives.
  - x^T and the v-columns of w_qkv are fused host-side into one "xw"
    tensor: one DMA per contraction tile (DMA-start overhead is a
    fixed cost per descriptor).
  - Output stored bf16 (halves store transfers incl. the tail-critical
    one); host converts back to f32. Adds ~0.2% RMS quantization --
    total rel err 6.0e-3 vs the 2e-2 gate.
"""

import os
import sys

for _p in ("/opt/trn_rl_repo", "/root/.axon_site/_ro/trn_rl_repo"):
    if os.path.isdir(_p) and _p not in sys.path:
        sys.path.insert(0, _p)
        break

import numpy as np
import ml_dtypes

import concourse.bass as bass
import concourse.bacc as bacc
import concourse.tile as tile
import concourse.mybir as mybir
from concourse import bass_utils

BF16 = mybir.dt.bfloat16
F32 = mybir.dt.float32
AF = mybir.ActivationFunctionType

B, N, C, H = 8, 1024, 1024, 16
D = C // H            # 64 head dim
P = 128               # partitions
CT = C // P           # 8 contraction tiles
NT2 = N // 512        # 2 n-tiles of 512
MT = N // P           # 8 m-tiles of 128
PAIRS = H // 2        # 8 head pairs
SCALE = float(D) ** -0.5
N_CORES = 8

_cache = {}


def _build():
    nc = bacc.Bacc("TRN2", target_bir_lowering=False, debug=False,
                   enable_asserts=False, num_devices=N_CORES)

    xw_d = nc.dram_tensor("xw", [C, 2 * N], BF16, kind="ExternalInput")
    wqkvT_d = nc.dram_tensor("wqkvT", [C, 3 * C], BF16, kind="ExternalInput")
    wprojT_d = nc.dram_tensor("wprojT", [C, C], BF16, kind="ExternalInput")
    bias_d = nc.dram_tensor("bias", [P, CT], F32, kind="ExternalInput")
    outT_d = nc.dram_tensor("outT", [C, N], BF16, kind="ExternalOutput")

    with tile.TileContext(nc) as tc:
        with (
            tc.tile_pool(name="res", bufs=1) as rp,
            tc.tile_pool(name="work", bufs=2) as wp,
            tc.tile_pool(name="ps", bufs=1, space="PSUM") as pp,
        ):
            # ---------------- PE warm-up ----------------
            # The PE sits idle ~3.5us waiting for the first input DMAs, and
            # the p-state/HAM ramp then penalizes the first ~3us of real
            # matmuls. Run dummy matmuls on memset data during the DMA wait
            # so the ramp completes before real work arrives.
            warm_a = wp.tile([P, 512], BF16, name="warm_a", tag="warm_a",
                             bufs=1)
            nc.gpsimd.memset(warm_a[:], 0.25)
            warm_ps = pp.tile([P, 512], F32, name="warm_ps", tag="acc",
                              bufs=3)
            for _ in range(6):
                nc.tensor.matmul(warm_ps[:], warm_a[:, 0:P], warm_a[:],
                                 start=True, stop=True)

            # ---------------- resident inputs ----------------
            # v-part inputs first so the first matmuls can start ASAP.
            # x and the v-columns of w_qkv are fused host-side into one
            # tensor: one DMA per c-tile (the HWDGE queue stage costs a
            # fixed ~625ns per DMA, so fewer DMAs = faster start)
            xT = []
            wqv = []
            for i in range(CT):
                t = rp.tile([P, 2 * N], BF16, name=f"xw{i}", tag=f"xw{i}")
                nc.sync.dma_start(t[:], xw_d.ap()[i * P:(i + 1) * P, :])
                xT.append(t[:, 0:N])
                wqv.append(t[:, N:2 * N])
            wqk = []
            for i in range(CT):
                t = rp.tile([P, 2 * C], BF16, name=f"wqk{i}", tag=f"wqk{i}")
                nc.sync.dma_start(t[:],
                                  wqkvT_d.ap()[i * P:(i + 1) * P, 0:2 * C])
                wqk.append(t)
            wpj = []
            for i in range(CT):
                t = rp.tile([P, C], BF16, name=f"wpj{i}", tag=f"wpj{i}")
                nc.sync.dma_start(t[:], wprojT_d.ap()[i * P:(i + 1) * P, :])
                wpj.append(t)
            bias_t = rp.tile([P, CT], F32, name="bias_t", tag="bias")
            nc.sync.dma_start(bias_t[:], bias_d.ap())

            # ---------------- result tiles ----------------
            qT = [rp.tile([P, N], BF16, name=f"qT{i}", tag=f"qT{i}")
                  for i in range(PAIRS)]
            kT = [rp.tile([P, N], BF16, name=f"kT{i}", tag=f"kT{i}")
                  for i in range(PAIRS)]
            vt = [[rp.tile([P, 8, D + 1], BF16, name=f"v{m}_{j}",
                           tag=f"v{m}_{j}") for j in range(2)]
                  for m in range(MT)]
            ao = [rp.tile([P, N], BF16, name=f"ao{i}", tag=f"ao{i}")
                  for i in range(PAIRS)]

            for m in range(MT):
                for j in range(2):
                    nc.vector.memset(vt[m][j][:, :, D:D + 1], 1.0)

            # ---------------- phase A(v): v projection ----------------
            # Alternate psum tags: the attention-phase "av" slots are idle
            # here, so borrow them for 4-deep accumulator pipelining.
            for m in range(MT):
                for j in range(2):
                    vtag, vbufs = (("acc", 3), ("av", 2),
                                   ("st", 3))[(2 * m + j) % 3]
                    ps = pp.tile([P, 512], F32, name=f"accv{m}_{j}", tag=vtag,
                                 bufs=vbufs)
                    for c in range(CT):
                        nc.tensor.matmul(
                            ps[:],
                            xT[c][:, m * P:(m + 1) * P],
                            wqv[c][:, j * 512:(j + 1) * 512],
                            start=(c == 0), stop=(c == CT - 1),
                        )
                    nc.vector.tensor_copy(
                        vt[m][j][:, :, 0:D],
                        ps[:].rearrange("p (h d) -> p h d", d=D),
                    )

            # ------- interleaved: q/k projection + attention per pair -------
            for pr in range(PAIRS):
                for which, dst in ((0, qT[pr]), (1, kT[pr])):
                    o0 = which * C + pr * P
                    for n2 in range(NT2):
                        nsl = slice(n2 * 512, (n2 + 1) * 512)
                        ps = pp.tile([P, 512], F32,
                                     name=f"accqk{pr}_{which}_{n2}",
                                     tag="acc", bufs=3)
                        for c in range(CT):
                            nc.tensor.matmul(
                                ps[:],
                                wqk[c][:, o0:o0 + P],
                                xT[c][:, nsl],
                                start=(c == 0), stop=(c == CT - 1),
                            )
                        nc.vector.tensor_copy(dst[:, nsl], ps[:])

                # attention for this pair (n2-outer so only one n2's AV
                # accumulators are live; PSUM budget: acc 3 + st 3 + av 2 = 8)
                for n2 in range(NT2):
                    nsl = slice(n2 * 512, (n2 + 1) * 512)
                    av = [pp.tile([D + 1, 512], F32, name=f"av{pr}_{n2}_{h}",
                                  tag="av", bufs=2) for h in range(2)]
                    for m in range(MT):
                        msl = slice(m * P, (m + 1) * P)
                        st = [pp.tile([P, 512], F32,
                                      name=f"st{pr}_{m}_{n2}_{h}", tag="st",
                                      bufs=3) for h in range(2)]
                        for h in range(2):
                            psl = slice(h * 64, (h + 1) * 64)
                            nc.tensor.matmul(
                                st[h][:],
                                kT[pr][psl, msl],
                                qT[pr][psl, nsl],
                                start=True, stop=True,
                                tile_position=(h * 64, 0),
                            )
                        for h in range(2):
                            pt = wp.tile([P, 512], BF16,
                                         name=f"pt{pr}_{m}_{n2}_{h}",
                                         tag="pt", bufs=6)
                            nc.scalar.activation(pt[:], st[h][:], AF.Exp,
                                                 scale=SCALE)
                            head = 2 * pr + h
                            vtile = vt[m][head // 8]
                            nc.tensor.matmul(
                                av[h][:],
                                vtile[:, head % 8, :],
                                pt[:],
                                start=(m == 0), stop=(m == MT - 1),
                            )
                    # normalize + evacuate this n2 slice. Copy PSUM out
                    # first so the av bank frees fast; normalize from SBUF.
                    # For the very last slice the bank release doesn't
                    # matter; read PSUM directly to shorten the tail chain.
                    last_slice = (pr == PAIRS - 1 and n2 == NT2 - 1)
                    for h in range(2):
                        if last_slice:
                            araw = av[h]
                        else:
                            araw = wp.tile([D + 1, 512], F32,
                                           name=f"araw{pr}_{h}_{n2}",
                                           tag="araw", bufs=4)
                            nc.vector.tensor_copy(araw[:], av[h][:])
                        rec = wp.tile([D + 1, 512], F32,
                                      name=f"rec{pr}_{h}_{n2}", tag="rec",
                                      bufs=4)
                        nc.vector.reciprocal(rec[D:D + 1, :],
                                             araw[D:D + 1, :])
                        rec0 = wp.tile([1, 512], F32,
                                       name=f"rec0_{pr}_{h}_{n2}", tag="rec0",
                                       bufs=4)
                        nc.sync.dma_start(rec0[:], rec[D:D + 1, :])
                        bc = wp.tile([D, 512], F32, name=f"bc{pr}_{h}_{n2}",
                                     tag="bc", bufs=4)
                        nc.gpsimd.partition_broadcast(bc[:], rec0[:])
                        if h == 0:
                            nc.vector.tensor_mul(ao[pr][0:D, nsl],
                                                 araw[0:D, :], bc[:])
                        else:
                            tmp = wp.tile([D, 512], BF16,
                                          name=f"aotmp{pr}_{n2}", tag="aotmp",
                                          bufs=4)
                            nc.vector.tensor_mul(tmp[:], araw[0:D, :],
                                                 bc[:])
                            nc.sync.dma_start(ao[pr][D:P, nsl], tmp[:])

            # ---------------- phase C: output projection ----------------
            # n2-outer: proj over n2=0 becomes ready while the last pair's
            # n2=1 attention still runs, filling the PE tail gap.
            for n2 in range(NT2):
                for ot in range(CT):
                    nsl = slice(n2 * 512, (n2 + 1) * 512)
                    # n2=1 runs at the very end when the attention's st/av
                    # slots are dead: spread the 8 chains across all tags so
                    # every pair-0..6 prelude can run before ao[7] arrives
                    if n2 == 0:
                        ptag, pbufs = "acc", 3
                    else:
                        ptag, pbufs = (("acc", 3), ("st", 3), ("av", 2),
                                       ("acc", 3), ("st", 3), ("av", 2),
                                       ("acc", 3), ("st", 3))[ot]
                    ps = pp.tile([P, 512], F32, name=f"accy{ot}_{n2}",
                                 tag=ptag, bufs=pbufs)
                    for pr in range(PAIRS):
                        nc.tensor.matmul(
                            ps[:],
                            wpj[pr][:, ot * P:(ot + 1) * P],
                            ao[pr][:, nsl],
                            start=(pr == 0), stop=(pr == PAIRS - 1),
                        )
                    yt = wp.tile([P, 512], BF16, name=f"y{ot}_{n2}", tag="y",
                                 bufs=3)
                    nc.vector.tensor_scalar_add(yt[:], ps[:],
                                                bias_t[:, ot:ot + 1])
                    nc.sync.dma_start(outT_d.ap()[ot * P:(ot + 1) * P, nsl],
                                      yt[:])

    nc.compile()
    return nc


def get_nc():
    if "nc" not in _cache:
        _cache["nc"] = _build()
    return _cache["nc"]


def kernel(x, w_qkv, w_proj, b_proj):
    x = np.asarray(x, dtype=np.float32)
    w_qkv = np.asarray(w_qkv, dtype=np.float32)
    w_proj = np.asarray(w_proj, dtype=np.float32)
    b_proj = np.asarray(b_proj, dtype=np.float32)

    bf = ml_dtypes.bfloat16
    wqkvT = np.ascontiguousarray(w_qkv.T).astype(bf)     # [C, 3C]
    wprojT = np.ascontiguousarray(w_proj.T).astype(bf)   # [C, C]
    bias = np.ascontiguousarray(b_proj.reshape(CT, P).T).astype(np.float32)

    in_maps = []
    wqv_host = wqkvT[:, 2 * C:]                          # [C, C] v columns
    for b in range(N_CORES):
        xT = np.ascontiguousarray(x[b].T).astype(bf)     # [C, N]
        xw = np.ascontiguousarray(np.concatenate([xT, wqv_host], axis=1))
        in_maps.append({"xw": xw, "wqkvT": wqkvT, "wprojT": wprojT,
                        "bias": bias})

    nc = get_nc()
    _cache["in_maps"] = in_maps
    res = bass_utils.run_bass_kernel_spmd(nc, in_maps,
                                          core_ids=list(range(N_CORES)))
    out = np.empty((B, N, C), dtype=np.float32)
    for b in range(N_CORES):
        out[b] = res.results[b]["outT"].T.astype(np.float32)
    return out



# revision 8
# speedup vs baseline: 1.0598x; 1.0598x over previous
"""Trainium2 Bass kernel for batched multi-head self-attention.

Reference computation (per batch element b):
    qkv = x @ w_qkv.T                  # [N, 3C]
    q, k, v = split/reshape to heads   # H=16 heads, d=64
    attn = softmax(q @ k.T / sqrt(d))
    out = (attn @ v) reshaped back     # [N, C]
    y = out @ w_proj.T + b_proj

Sharding: pure data-parallel over batch B=8 across the 8 NeuronCores
(one batch element per core, weights replicated, no collectives).

On-device layout (everything transposed so matmuls contract over the
partition axis with no input transposes):
  - xT      [C, N]   (host pre-transposed, bf16)
  - wqkT    [C, 2C]  (host pre-transposed q,k columns, bf16)
  - wprojT  [C, C]   (host pre-transposed, bf16)

Cost model: a PE matmul costs out_free_size * 0.4167ns regardless of
contraction size or output partitions.  The key restructure vs the
229us baseline is the attn@V matmul orientation:
  - scores S^T [m,n] tiles (K=64, head pairs row-packed): free=512.
  - AV is computed FLIPPED: out[n, d+1] = pt[m, n-tile].T @ [V | 1]
    with free=65 instead of the old [d+1, 512] free=512 layout.
    This halves AV PE time (54.6us -> 27.7us).  The ones column gives
    softmax row sums per output partition, so normalization is a
    per-partition tensor_scalar divide on DVE (no partition
    broadcasts).
  - The [n, c] result is transposed back to [c, n] for the output
    projection with PE transpose instructions (128x128 via identity,
    53ns each, 64 total = 3.4us).
  - exp() is done on [128, 1024] head-pair tiles (one ACT instruction
    per (n2, m)) to amortize the ~185ns ACT access-latency overhead;
    ACT total 133us stays under the PE's ~197us.
Emission interleaves each attention slot (pr, n2) with the previous
slot's AV/transpose work and the next pair's q/k projection chains so
the PE never waits on ACT; the output projection fills the tail.
PSUM budget (8 banks): st 2x2 + acc 1 + av 2 + tr 1.
"""

import os
import sys

for _p in ("/opt/trn_rl_repo", "/root/.axon_site/_ro/trn_rl_repo"):
    if os.path.isdir(_p) and _p not in sys.path:
        sys.path.insert(0, _p)
        break

import numpy as np
import ml_dtypes

import concourse.bass as bass
import concourse.bacc as bacc
import concourse.tile as tile
import concourse.mybir as mybir
from concourse import bass_utils

BF16 = mybir.dt.bfloat16
F32 = mybir.dt.float32
AF = mybir.ActivationFunctionType
ALU = mybir.AluOpType

B, N, C, H = 8, 1024, 1024, 16
D = C // H            # 64 head dim
P = 128               # partitions
CT = C // P           # 8 contraction tiles
NT2 = N // 512        # 2 n-tiles of 512
MT = N // P           # 8 m-tiles of 128
PAIRS = H // 2        # 8 head pairs
SCALE = float(D) ** -0.5
N_CORES = 8

_cache = {}


def _build():
    nc = bacc.Bacc("TRN2", target_bir_lowering=False, debug=False,
                   enable_asserts=False, num_devices=N_CORES)

    xw_d = nc.dram_tensor("xw", [C, 2 * N], BF16, kind="ExternalInput")
    wqkT_d = nc.dram_tensor("wqkT", [C, 2 * C], BF16, kind="ExternalInput")
    wprojT_d = nc.dram_tensor("wprojT", [C, C], BF16, kind="ExternalInput")
    bias_d = nc.dram_tensor("bias", [P, CT], F32, kind="ExternalInput")
    ident_d = nc.dram_tensor("ident", [P, P], BF16, kind="ExternalInput")
    outT_d = nc.dram_tensor("outT", [C, N], BF16, kind="ExternalOutput")

    with tile.TileContext(nc) as tc:
        with (
            tc.tile_pool(name="res", bufs=1) as rp,
            tc.tile_pool(name="work", bufs=2) as wp,
            tc.tile_pool(name="ps", bufs=1, space="PSUM") as pp,
        ):
            # ---------------- PE warm-up ----------------
            # Cover the initial input-DMA wait with dummy matmuls so the
            # p-state/HAM ramp completes before real work arrives.
            warm_a = wp.tile([P, 512], BF16, name="warm_a", tag="warm_a",
                             bufs=1)
            nc.gpsimd.memset(warm_a[:], 0.25)
            warm_ps = pp.tile([P, 512], F32, name="warm_ps", tag="acc",
                              bufs=1)
            for _ in range(6):
                nc.tensor.matmul(warm_ps[:], warm_a[:, 0:P], warm_a[:],
                                 start=True, stop=True)

            # ---------------- resident inputs ----------------
            xT = []
            wqv = []
            for i in range(CT):
                t = rp.tile([P, 2 * N], BF16, name=f"xw{i}", tag=f"xw{i}")
                nc.sync.dma_start(t[:], xw_d.ap()[i * P:(i + 1) * P, :])
                xT.append(t[:, 0:N])
                wqv.append(t[:, N:2 * N])
            wqk = []
            for i in range(CT):
                t = rp.tile([P, 2 * C], BF16, name=f"wqk{i}", tag=f"wqk{i}")
                nc.sync.dma_start(t[:], wqkT_d.ap()[i * P:(i + 1) * P, :])
                wqk.append(t)
            wpj = []
            for i in range(CT):
                t = rp.tile([P, C], BF16, name=f"wpj{i}", tag=f"wpj{i}")
                nc.sync.dma_start(t[:], wprojT_d.ap()[i * P:(i + 1) * P, :])
                wpj.append(t)
            bias_t = rp.tile([P, CT], F32, name="bias_t", tag="bias")
            nc.sync.dma_start(bias_t[:], bias_d.ap())
            ident_t = rp.tile([P, P], BF16, name="ident_t", tag="ident")
            nc.sync.dma_start(ident_t[:], ident_d.ap())

            # ---------------- result tiles ----------------
            qT = [rp.tile([P, N], BF16, name=f"qT{i}", tag=f"qT{i}")
                  for i in range(PAIRS)]
            kT = [rp.tile([P, N], BF16, name=f"kT{i}", tag=f"kT{i}")
                  for i in range(PAIRS)]
            # vt[m]: [128 m-rows, 16 heads, 64 v-dims + ones col]
            vt = [rp.tile([P, H, D + 1], BF16, name=f"vt{m}", tag=f"vt{m}")
                  for m in range(MT)]
            ao = [rp.tile([P, N], BF16, name=f"ao{i}", tag=f"ao{i}")
                  for i in range(PAIRS)]

            for m in range(MT):
                nc.vector.memset(vt[m][:, :, D:D + 1], 1.0)

            # ---------------- emission helpers ----------------
            def vproj_m(m):
                """v-projection for m-tile m: [128 tokens, 1024 v-dims]."""
                ps = pp.tile([P, 1024], F32, name=f"vps{m}", tag="st", bufs=2)
                for j in range(2):
                    for c in range(CT):
                        nc.tensor.matmul(
                            ps[:, j * 512:(j + 1) * 512],
                            xT[c][:, m * P:(m + 1) * P],
                            wqv[c][:, j * 512:(j + 1) * 512],
                            start=(c == 0), stop=(c == CT - 1),
                        )
                nc.vector.tensor_copy(
                    vt[m][:, :, 0:D],
                    ps[:].rearrange("p (h d) -> p h d", d=D),
                )

            def qk_chain_mms(pr, which, n2, tag):
                """Returns (list of mm closures, finish closure)."""
                o0 = which * C + pr * P
                nsl = slice(n2 * 512, (n2 + 1) * 512)
                ps = pp.tile([P, 512], F32, name=f"qk{pr}_{which}_{n2}",
                             tag=tag, bufs=1)
                dst = (qT if which == 0 else kT)[pr]

                def mk(c):
                    def go():
                        nc.tensor.matmul(
                            ps[:],
                            wqk[c][:, o0:o0 + P],
                            xT[c][:, nsl],
                            start=(c == 0), stop=(c == CT - 1),
                        )
                    return go

                def fin():
                    nc.vector.tensor_copy(dst[:, nsl], ps[:])

                return [mk(c) for c in range(CT)], fin

            def s_pair(pr, n2, m):
                """Score matmuls for both heads of the pair + combined exp.
                Returns the pt tile."""
                nsl = slice(n2 * 512, (n2 + 1) * 512)
                msl = slice(m * P, (m + 1) * P)
                st_t = pp.tile([P, 1024], F32, name=f"st{pr}_{n2}_{m}",
                               tag="st", bufs=2)
                for h in range(2):
                    psl = slice(h * 64, (h + 1) * 64)
                    nc.tensor.matmul(
                        st_t[:, h * 512:(h + 1) * 512],
                        kT[pr][psl, msl],
                        qT[pr][psl, nsl],
                        start=True, stop=True,
                        tile_position=(h * 64, 0),
                    )
                pt_t = wp.tile([P, 1024], BF16, name=f"pt{pr}_{n2}_{m}",
                               tag="pt", bufs=18)
                nc.scalar.activation(pt_t[:], st_t[:], AF.Exp, scale=SCALE)
                return pt_t

            def av_chain(pr, n2, h, nu, pts, an_t):
                """Flipped AV for one head and one 128-col n-tile."""
                head = 2 * pr + h
                av_t = pp.tile([P, D + 1], F32, name=f"av{pr}_{n2}_{h}_{nu}",
                               tag="av", bufs=2)
                lo = h * 512 + nu * 128
                for mi in range(MT):
                    nc.tensor.matmul(
                        av_t[:],
                        pts[mi][:, lo:lo + 128],
                        vt[mi][:, head, :],
                        start=(mi == 0), stop=(mi == MT - 1),
                    )
                # normalize + evacuate: per-partition scale by 1/sums col
                rec = wp.tile([P, 1], F32, name=f"rc{pr}_{n2}_{h}_{nu}",
                              tag="rec", bufs=4)
                nc.vector.reciprocal(rec[:], av_t[:, D:D + 1])
                nc.vector.tensor_scalar_mul(
                    an_t[:, h * 64:(h + 1) * 64], av_t[:, 0:D], rec[:])

            def proj_chain_mms(n2, ot, ps):
                """Output projection chain closures for tile (n2, ot)."""
                nsl = slice(n2 * 512, (n2 + 1) * 512)

                def mk(pr):
                    def go():
                        nc.tensor.matmul(
                            ps[:],
                            wpj[pr][:, ot * P:(ot + 1) * P],
                            ao[pr][:, nsl],
                            start=(pr == 0), stop=(pr == PAIRS - 1),
                        )
                    return go

                def fin():
                    yt = wp.tile([P, 512], BF16, name=f"y{ot}_{n2}", tag="y",
                                 bufs=3)
                    nc.vector.tensor_scalar_add(yt[:], ps[:],
                                                bias_t[:, ot:ot + 1])
                    nc.sync.dma_start(outT_d.ap()[ot * P:(ot + 1) * P, nsl],
                                      yt[:])

                return [mk(pr) for pr in range(PAIRS)], fin

            # ---------------- startup ----------------
            # vproj m0..m5 interleaved with the 4 qk chains of pair 0
            # (wqk DMAs land after xw, so qk starts late on purpose).
            qk0 = []
            for which, n2, tag in ((0, 0, "acc"), (0, 1, "tr"),
                                   (1, 0, "acc"), (1, 1, "tr")):
                qk0.append(qk_chain_mms(0, which, n2, tag))
            vproj_m(0)
            vproj_m(1)
            for i, (mms, fin) in enumerate(qk0):
                for go in mms:
                    go()
                fin()
                if i + 2 < 6:
                    vproj_m(i + 2)

            # ---------------- main loop ----------------
            # Slot (pr, n2).  Filler work per slot:
            #  - previous slot's AV chains + divides + transposes + ao evac
            #  - next pair's q (n2=0 slot) / k (n2=1 slot) projection chains
            #  - vproj m6/m7 in slot (0,0); output projection at pr=7
            pts_prev = None      # (pr, n2, [pt tiles]) of previous slot

            for pr in range(PAIRS):
                for n2 in range(NT2):
                    slot = 2 * pr + n2
                    # --- gather filler: qk chains of next pair ---
                    qk_fill = []
                    if pr < PAIRS - 2:
                        which = n2          # q chains in n2=0, k in n2=1
                        qk_fill.append(
                            qk_chain_mms(pr + 1, which, 0, "acc"))
                        qk_fill.append(
                            qk_chain_mms(pr + 1, which, 1, "tr"))
                    elif pr == PAIRS - 2:
                        # spread pair-7 chains: q both in (6,0), kA in (6,1)
                        if n2 == 0:
                            qk_fill.append(qk_chain_mms(7, 0, 0, "acc"))
                            qk_fill.append(qk_chain_mms(7, 0, 1, "tr"))
                        else:
                            qk_fill.append(qk_chain_mms(7, 1, 0, "acc"))
                    elif pr == PAIRS - 1 and n2 == 0:
                        qk_fill.append(qk_chain_mms(7, 1, 1, "acc"))

                    # --- previous slot's AV work ---
                    if pts_prev is not None:
                        ppr, pn2, ppts = pts_prev
                        an_ts = [wp.tile([P, P], BF16,
                                         name=f"an{ppr}_{pn2}_{nu}",
                                         tag="an", bufs=6)
                                 for nu in range(4)]
                        tr_t = pp.tile([P, 512], BF16,
                                       name=f"tr{ppr}_{pn2}", tag="tr",
                                       bufs=1)

                        def mk_av(nu, h, _ppr=ppr, _pn2=pn2, _ppts=ppts,
                                  _an=an_ts):
                            def go():
                                av_chain(_ppr, _pn2, h, nu, _ppts, _an[nu])
                            return go

                        def mk_tr(nu, _an=an_ts, _tr=tr_t):
                            def go():
                                nc.tensor.transpose(
                                    _tr[:, nu * 128:(nu + 1) * 128],
                                    _an[nu][:], ident_t[:])
                            return go

                        def mk_evac(_ppr=ppr, _pn2=pn2, _tr=tr_t):
                            def go():
                                nc.vector.tensor_copy(
                                    ao[_ppr][:, _pn2 * 512:(_pn2 + 1) * 512],
                                    _tr[:])
                            return go

                        av_items = [mk_av(nu, h)
                                    for nu in range(4) for h in range(2)]
                        tr_items = [mk_tr(nu) for nu in range(4)]
                        evac_item = mk_evac()
                    else:
                        av_items, tr_items, evac_item = [], [], None

                    # --- weave the slot ---
                    # filler queue: list of closure-lists, consumed in order
                    # across the 8 m-steps.
                    fq = []
                    if slot == 0:
                        # vproj m6/m7 split across the early m-steps
                        fq.extend([[lambda m=6: vproj_m(m)],
                                   [lambda m=7: vproj_m(m)]])
                    for mms, fin in qk_fill:
                        def qk_part(items):
                            def go():
                                for it in items:
                                    it()
                            return go
                        fq.append([qk_part(mms[0:4])])
                        fin_ = fin

                        def qk_rest(items=mms[4:8], f=fin_):
                            def go():
                                for it in items:
                                    it()
                                f()
                            return go
                        fq.append([qk_rest()])
                    # AV chains spread over mid/late m-steps, transposes after
                    av_sched = {3: av_items[0:2], 4: av_items[2:4],
                                5: av_items[4:6], 6: av_items[6:8]}
                    tr_sched = {5: tr_items[0:1], 6: tr_items[1:2],
                                7: tr_items[2:4]}

                    pts_now = []
                    for m in range(MT):
                        pts_now.append(s_pair(pr, n2, m))
                        if m < len(fq):
                            for it in fq[m]:
                                it()
                        for it in av_sched.get(m, []):
                            it()
                        for it in tr_sched.get(m, []):
                            it()
                        if m == MT - 1:
                            # leftover filler (slots with >8 filler groups)
                            for grp in fq[MT:]:
                                for it in grp:
                                    it()
                            if evac_item is not None:
                                evac_item()
                    pts_prev = (pr, n2, pts_now)

            # ---------------- tail ----------------
            # last slot's AV + transposes, then the rest of the projection
            ppr, pn2, ppts = pts_prev
            an_ts = [wp.tile([P, P], BF16, name=f"an{ppr}_{pn2}_{nu}",
                             tag="an", bufs=6) for nu in range(4)]
            tr_t = pp.tile([P, 512], BF16, name=f"tr{ppr}_{pn2}", tag="tr",
                           bufs=1)

            # fill the E(7,1,7) wait with two n2=0 proj chains on st halves
            st_tail = pp.tile([P, 1024], F32, name="st_tail", tag="st",
                              bufs=2)
            tail_fins = []
            for j, ot in enumerate((0, 1)):
                mms, fin = proj_chain_mms(0, ot,
                                          st_tail[:, j * 512:(j + 1) * 512])
                for go in mms[0:4]:
                    go()
                tail_fins.append((mms[4:], fin))

            for nu in range(4):
                for h in range(2):
                    av_chain(ppr, pn2, h, nu, ppts, an_ts[nu])
                if nu >= 1 and tail_fins:
                    mms, fin = tail_fins.pop(0)
                    for go in mms:
                        go()
                    fin()
            for mms, fin in tail_fins:
                for go in mms:
                    go()
                fin()
            for nu in range(4):
                nc.tensor.transpose(tr_t[:, nu * 128:(nu + 1) * 128],
                                    an_ts[nu][:], ident_t[:])
            nc.vector.tensor_copy(ao[ppr][:, pn2 * 512:(pn2 + 1) * 512],
                                  tr_t[:])

            # remaining projection: n2=0 ot=2..7, n2=1 ot=0..7
            remaining = [(0, ot) for ot in range(2, CT)]
            remaining += [(1, ot) for ot in range(CT)]
            tags = ["st2", "st2", "st3", "st3", "acc", "tr", "av", "av"]
            st2 = pp.tile([P, 1024], F32, name="st2", tag="st", bufs=2)
            st3 = pp.tile([P, 1024], F32, name="st3", tag="st", bufs=2)
            fins = []
            for i, (n2, ot) in enumerate(remaining):
                tg = tags[i % 8]
                if tg == "st2":
                    ps = st2[:, (i % 2) * 512:((i % 2) + 1) * 512]
                elif tg == "st3":
                    ps = st3[:, (i % 2) * 512:((i % 2) + 1) * 512]
                elif tg == "av":
                    ps = pp.tile([P, 512], F32, name=f"ytail{i}", tag="av",
                                 bufs=2)
                else:
                    ps = pp.tile([P, 512], F32, name=f"ytail{i}", tag=tg,
                                 bufs=1)
                mms, fin = proj_chain_mms(n2, ot, ps)
                for go in mms:
                    go()
                fins.append(fin)
                # drain finishes with one-chain delay so psum bufs recycle
                if len(fins) >= 2:
                    fins.pop(0)()
            for fin in fins:
                fin()

    nc.compile()
    return nc


def get_nc():
    if "nc" not in _cache:
        _cache["nc"] = _build()
    return _cache["nc"]


def kernel(x, w_qkv, w_proj, b_proj):
    x = np.asarray(x, dtype=np.float32)
    w_qkv = np.asarray(w_qkv, dtype=np.float32)
    w_proj = np.asarray(w_proj, dtype=np.float32)
    b_proj = np.asarray(b_proj, dtype=np.float32)

    bf = ml_dtypes.bfloat16
    wqkvT = np.ascontiguousarray(w_qkv.T).astype(bf)     # [C, 3C]
    wqkT = np.ascontiguousarray(wqkvT[:, 0:2 * C])       # [C, 2C] q,k cols
    wprojT = np.ascontiguousarray(w_proj.T).astype(bf)   # [C, C]
    bias = np.ascontiguousarray(b_proj.reshape(CT, P).T).astype(np.float32)
    ident = np.eye(P, dtype=bf)

    in_maps = []
    wqv_host = wqkvT[:, 2 * C:]                          # [C, C] v columns
    for b in range(N_CORES):
        xT = np.ascontiguousarray(x[b].T).astype(bf)     # [C, N]
        xw = np.ascontiguousarray(np.concatenate([xT, wqv_host], axis=1))
        in_maps.append({"xw": xw, "wqkT": wqkT, "wprojT": wprojT,
                        "bias": bias, "ident": ident})

    nc = get_nc()
    _cache["in_maps"] = in_maps
    res = bass_utils.run_bass_kernel_spmd(nc, in_maps,
                                          core_ids=list(range(N_CORES)))
    out = np.empty((B, N, C), dtype=np.float32)
    for b in range(N_CORES):
        out[b] = res.results[b]["outT"].T.astype(np.float32)
    return out


# revision 12
# speedup vs baseline: 1.0919x; 1.0303x over previous
"""Trainium2 Bass kernel for batched multi-head self-attention.

Reference computation (per batch element b):
    qkv = x @ w_qkv.T                  # [N, 3C]
    q, k, v = split/reshape to heads   # H=16 heads, d=64
    attn = softmax(q @ k.T / sqrt(d))
    out = (attn @ v) reshaped back     # [N, C]
    y = out @ w_proj.T + b_proj

Sharding: pure data-parallel over batch B=8 across the 8 NeuronCores
(one batch element per core, weights replicated, no collectives).

On-device layout (everything transposed so matmuls contract over the
partition axis with no input transposes):
  - xT      [C, N]   (host pre-transposed, bf16)
  - wqkT    [C, 2C]  (host pre-transposed q,k columns, bf16)
  - wprojT  [C, C]   (host pre-transposed, bf16)

Cost model: a PE matmul costs out_free_size * 0.4167ns regardless of
contraction size or output partitions.  The key restructure vs the
229us baseline is the attn@V matmul orientation:
  - scores S^T [m,n] tiles (K=64, head pairs row-packed): free=512.
  - AV is computed FLIPPED: out[n, d+1] = pt[m, n-tile].T @ [V | 1]
    with free=65 instead of the old [d+1, 512] free=512 layout.
    This halves AV PE time (54.6us -> 27.7us).  The ones column gives
    softmax row sums per output partition, so normalization is a
    per-partition tensor_scalar divide on DVE (no partition
    broadcasts).
  - The [n, c] result is transposed back to [c, n] for the output
    projection with PE transpose instructions (128x128 via identity,
    53ns each, 64 total = 3.4us).
  - exp() is done on [128, 1024] head-pair tiles (one ACT instruction
    per (n2, m)) to amortize the ~185ns ACT access-latency overhead;
    ACT total 133us stays under the PE's ~197us.
Emission interleaves each attention slot (pr, n2) with the previous
slot's AV/transpose work and the next pair's q/k projection chains so
the PE never waits on ACT; the output projection fills the tail.
PSUM budget (8 banks): st 2x2 + acc 1 + av 2 + tr 1.
"""

import os
import sys

for _p in ("/opt/trn_rl_repo", "/root/.axon_site/_ro/trn_rl_repo"):
    if os.path.isdir(_p) and _p not in sys.path:
        sys.path.insert(0, _p)
        break

import numpy as np
import ml_dtypes

import concourse.bass as bass
import concourse.bacc as bacc
import concourse.tile as tile
import concourse.mybir as mybir
from concourse import bass_utils

BF16 = mybir.dt.bfloat16
F32 = mybir.dt.float32
AF = mybir.ActivationFunctionType
ALU = mybir.AluOpType

B, N, C, H = 8, 1024, 1024, 16
D = C // H            # 64 head dim
P = 128               # partitions
CT = C // P           # 8 contraction tiles
NT2 = N // 512        # 2 n-tiles of 512
MT = N // P           # 8 m-tiles of 128
PAIRS = H // 2        # 8 head pairs
SCALE = float(D) ** -0.5
N_CORES = 8

_cache = {}


def _build():
    nc = bacc.Bacc("TRN2", target_bir_lowering=False, debug=False,
                   enable_asserts=False, num_devices=N_CORES)

    xw_d = nc.dram_tensor("xw", [C, 2 * N], BF16, kind="ExternalInput")
    wqkT_d = nc.dram_tensor("wqkT", [C, 2 * C], BF16, kind="ExternalInput")
    wprojT_d = nc.dram_tensor("wprojT", [C, C], BF16, kind="ExternalInput")
    bias_d = nc.dram_tensor("bias", [P, CT], F32, kind="ExternalInput")
    ident_d = nc.dram_tensor("ident", [P, P], BF16, kind="ExternalInput")
    outT_d = nc.dram_tensor("outT", [C, N], BF16, kind="ExternalOutput")

    with tile.TileContext(nc) as tc:
        with (
            tc.tile_pool(name="res", bufs=1) as rp,
            tc.tile_pool(name="work", bufs=2) as wp,
            tc.tile_pool(name="ps", bufs=1, space="PSUM") as pp,
        ):
            # ---------------- PE warm-up ----------------
            # Cover the initial input-DMA wait with dummy matmuls so the
            # p-state/HAM ramp completes before real work arrives.
            warm_a = wp.tile([P, 512], BF16, name="warm_a", tag="warm_a",
                             bufs=1)
            nc.gpsimd.memset(warm_a[:], 0.25)
            warm_ps = pp.tile([P, 512], F32, name="warm_ps", tag="acc",
                              bufs=1)
            for _ in range(6):
                nc.tensor.matmul(warm_ps[:], warm_a[:, 0:P], warm_a[:],
                                 start=True, stop=True)

            # ---------------- resident inputs ----------------
            xT = []
            wqv = []
            for i in range(CT):
                t = rp.tile([P, 2 * N], BF16, name=f"xw{i}", tag=f"xw{i}")
                nc.sync.dma_start(t[:], xw_d.ap()[i * P:(i + 1) * P, :])
                xT.append(t[:, 0:N])
                wqv.append(t[:, N:2 * N])
            wqk = []
            for i in range(CT):
                t = rp.tile([P, 2 * C], BF16, name=f"wqk{i}", tag=f"wqk{i}")
                nc.sync.dma_start(t[:], wqkT_d.ap()[i * P:(i + 1) * P, :])
                wqk.append(t)
            wpj = []
            for i in range(CT):
                t = rp.tile([P, C], BF16, name=f"wpj{i}", tag=f"wpj{i}")
                nc.sync.dma_start(t[:], wprojT_d.ap()[i * P:(i + 1) * P, :])
                wpj.append(t)
            bias_t = rp.tile([P, CT], F32, name="bias_t", tag="bias")
            nc.sync.dma_start(bias_t[:], bias_d.ap())
            ident_t = rp.tile([P, P], BF16, name="ident_t", tag="ident")
            nc.sync.dma_start(ident_t[:], ident_d.ap())

            # ---------------- result tiles ----------------
            qT = [rp.tile([P, N], BF16, name=f"qT{i}", tag=f"qT{i}")
                  for i in range(PAIRS)]
            kT = [rp.tile([P, N], BF16, name=f"kT{i}", tag=f"kT{i}")
                  for i in range(PAIRS)]
            # vt[m]: [128 m-rows, 16 heads, 64 v-dims + ones col]
            vt = [rp.tile([P, H, D + 1], BF16, name=f"vt{m}", tag=f"vt{m}")
                  for m in range(MT)]
            ao = [rp.tile([P, N], BF16, name=f"ao{i}", tag=f"ao{i}")
                  for i in range(PAIRS)]

            for m in range(MT):
                nc.vector.memset(vt[m][:, :, D:D + 1], 1.0)

            # ---------------- emission helpers ----------------
            def vproj_m(m):
                """v-projection for m-tile m: [128 tokens, 1024 v-dims]."""
                ps = pp.tile([P, 1024], F32, name=f"vps{m}", tag="st", bufs=2)
                for j in range(2):
                    for c in range(CT):
                        nc.tensor.matmul(
                            ps[:, j * 512:(j + 1) * 512],
                            xT[c][:, m * P:(m + 1) * P],
                            wqv[c][:, j * 512:(j + 1) * 512],
                            start=(c == 0), stop=(c == CT - 1),
                        )
                nc.vector.tensor_copy(
                    vt[m][:, :, 0:D],
                    ps[:].rearrange("p (h d) -> p h d", d=D),
                )

            def qk_chain_mms(pr, which, n2, tag):
                """Returns (list of mm closures, finish closure)."""
                o0 = which * C + pr * P
                nsl = slice(n2 * 512, (n2 + 1) * 512)
                ps = pp.tile([P, 512], F32, name=f"qk{pr}_{which}_{n2}",
                             tag=tag, bufs=1)
                dst = (qT if which == 0 else kT)[pr]

                def mk(c):
                    def go():
                        nc.tensor.matmul(
                            ps[:],
                            wqk[c][:, o0:o0 + P],
                            xT[c][:, nsl],
                            start=(c == 0), stop=(c == CT - 1),
                        )
                    return go

                def fin():
                    nc.vector.tensor_copy(dst[:, nsl], ps[:])

                return [mk(c) for c in range(CT)], fin

            def s_pair(pr, n2, m):
                """Score matmuls for both heads of the pair + combined exp.
                Returns the pt tile."""
                nsl = slice(n2 * 512, (n2 + 1) * 512)
                msl = slice(m * P, (m + 1) * P)
                st_t = pp.tile([P, 1024], F32, name=f"st{pr}_{n2}_{m}",
                               tag="st", bufs=2)
                for h in range(2):
                    psl = slice(h * 64, (h + 1) * 64)
                    nc.tensor.matmul(
                        st_t[:, h * 512:(h + 1) * 512],
                        kT[pr][psl, msl],
                        qT[pr][psl, nsl],
                        start=True, stop=True,
                        tile_position=(h * 64, 0),
                    )
                pt_t = wp.tile([P, 1024], BF16, name=f"pt{pr}_{n2}_{m}",
                               tag="pt", bufs=18)
                nc.scalar.activation(pt_t[:], st_t[:], AF.Exp, scale=SCALE)
                return pt_t

            def av_chain(pr, n2, h, nu, pts, an_t):
                """Flipped AV for one head and one 128-col n-tile."""
                head = 2 * pr + h
                av_t = pp.tile([P, D + 1], F32, name=f"av{pr}_{n2}_{h}_{nu}",
                               tag="av", bufs=2)
                lo = h * 512 + nu * 128
                for mi in range(MT):
                    nc.tensor.matmul(
                        av_t[:],
                        pts[mi][:, lo:lo + 128],
                        vt[mi][:, head, :],
                        start=(mi == 0), stop=(mi == MT - 1),
                    )
                # normalize + evacuate: per-partition scale by 1/sums col
                rec = wp.tile([P, 1], F32, name=f"rc{pr}_{n2}_{h}_{nu}",
                              tag="rec", bufs=4)
                nc.vector.reciprocal(rec[:], av_t[:, D:D + 1])
                nc.vector.tensor_scalar_mul(
                    an_t[:, h * 64:(h + 1) * 64], av_t[:, 0:D], rec[:])

            def proj_chain_mms(n2, ot, ps):
                """Output projection chain closures for tile (n2, ot)."""
                nsl = slice(n2 * 512, (n2 + 1) * 512)

                def mk(pr):
                    def go():
                        nc.tensor.matmul(
                            ps[:],
                            wpj[pr][:, ot * P:(ot + 1) * P],
                            ao[pr][:, nsl],
                            start=(pr == 0), stop=(pr == PAIRS - 1),
                        )
                    return go

                def fin():
                    yt = wp.tile([P, 512], BF16, name=f"y{ot}_{n2}", tag="y",
                                 bufs=3)
                    nc.vector.tensor_scalar_add(yt[:], ps[:],
                                                bias_t[:, ot:ot + 1])
                    nc.sync.dma_start(outT_d.ap()[ot * P:(ot + 1) * P, nsl],
                                      yt[:])

                return [mk(pr) for pr in range(PAIRS)], fin

            # ---------------- startup ----------------
            # Phase 1: vproj m0..m3 c-OUTER with 8 simultaneously-open psum
            # chains (all 8 banks) so each arriving xw c-tile feeds 8 matmuls
            # immediately -- the PE tracks the DMA stream instead of stalling
            # for the full 4MB xw tensor.
            ps01 = [pp.tile([P, 1024], F32, name=f"vps{m}", tag="st", bufs=2)
                    for m in range(2)]
            ps23 = {(2, 0): pp.tile([P, 512], F32, name="vp2a", tag="acc",
                                    bufs=1),
                    (2, 1): pp.tile([P, 512], F32, name="vp2b", tag="tr",
                                    bufs=1),
                    (3, 0): pp.tile([P, 512], F32, name="vp3a", tag="av",
                                    bufs=2),
                    (3, 1): pp.tile([P, 512], F32, name="vp3b", tag="av",
                                    bufs=2)}
            for c in range(CT):
                for m in range(4):
                    for j in range(2):
                        dst = (ps01[m][:, j * 512:(j + 1) * 512] if m < 2
                               else ps23[(m, j)][:])
                        nc.tensor.matmul(
                            dst,
                            xT[c][:, m * P:(m + 1) * P],
                            wqv[c][:, j * 512:(j + 1) * 512],
                            start=(c == 0), stop=(c == CT - 1),
                        )
            for m in range(2):
                nc.vector.tensor_copy(
                    vt[m][:, :, 0:D],
                    ps01[m][:].rearrange("p (h d) -> p h d", d=D))
            for m in (2, 3):
                for j in range(2):
                    nc.vector.tensor_copy(
                        vt[m][:, j * 8:(j + 1) * 8, 0:D],
                        ps23[(m, j)][:].rearrange("p (h d) -> p h d", d=D))

            # Phase 2: vproj m4..m7 as m-chains, then the pair-0 qk chains
            # (their wqk DMAs land only after all of xw).
            for m in range(4, MT):
                vproj_m(m)
            for which, n2, tag in ((0, 0, "acc"), (0, 1, "tr"),
                                   (1, 0, "acc"), (1, 1, "tr")):
                mms, fin = qk_chain_mms(0, which, n2, tag)
                for go in mms:
                    go()
                fin()

            # ---------------- main loop ----------------
            # Slot (pr, n2).  Filler work per slot:
            #  - previous slot's AV chains + divides + transposes + ao evac
            #  - next pair's q (n2=0 slot) / k (n2=1 slot) projection chains
            #  - vproj m6/m7 in slot (0,0); output projection at pr=7
            pts_prev = None      # (pr, n2, [pt tiles]) of previous slot

            for pr in range(PAIRS):
                for n2 in range(NT2):
                    slot = 2 * pr + n2
                    # --- gather filler: qk chains of next pair ---
                    qk_fill = []
                    if pr < PAIRS - 2:
                        which = n2          # q chains in n2=0, k in n2=1
                        qk_fill.append(
                            qk_chain_mms(pr + 1, which, 0, "acc"))
                        qk_fill.append(
                            qk_chain_mms(pr + 1, which, 1, "tr"))
                    elif pr == PAIRS - 2:
                        # spread pair-7 chains: q both in (6,0), kA in (6,1)
                        if n2 == 0:
                            qk_fill.append(qk_chain_mms(7, 0, 0, "acc"))
                            qk_fill.append(qk_chain_mms(7, 0, 1, "tr"))
                        else:
                            qk_fill.append(qk_chain_mms(7, 1, 0, "acc"))
                    elif pr == PAIRS - 1 and n2 == 0:
                        qk_fill.append(qk_chain_mms(7, 1, 1, "acc"))

                    # --- previous slot's AV work ---
                    if pts_prev is not None:
                        ppr, pn2, ppts = pts_prev
                        an_ts = [wp.tile([P, P], BF16,
                                         name=f"an{ppr}_{pn2}_{nu}",
                                         tag="an", bufs=6)
                                 for nu in range(4)]
                        tr_t = pp.tile([P, 512], BF16,
                                       name=f"tr{ppr}_{pn2}", tag="tr",
                                       bufs=1)

                        def mk_av(nu, h, _ppr=ppr, _pn2=pn2, _ppts=ppts,
                                  _an=an_ts):
                            def go():
                                av_chain(_ppr, _pn2, h, nu, _ppts, _an[nu])
                            return go

                        def mk_tr(nu, _an=an_ts, _tr=tr_t):
                            def go():
                                nc.tensor.transpose(
                                    _tr[:, nu * 128:(nu + 1) * 128],
                                    _an[nu][:], ident_t[:])
                            return go

                        def mk_evac(_ppr=ppr, _pn2=pn2, _tr=tr_t):
                            def go():
                                nc.vector.tensor_copy(
                                    ao[_ppr][:, _pn2 * 512:(_pn2 + 1) * 512],
                                    _tr[:])
                            return go

                        av_items = [mk_av(nu, h)
                                    for nu in range(4) for h in range(2)]
                        tr_items = [mk_tr(nu) for nu in range(4)]
                        evac_item = mk_evac()
                    else:
                        av_items, tr_items, evac_item = [], [], None

                    # --- weave the slot ---
                    # filler queue: list of closure-lists, consumed in order
                    # across the 8 m-steps.
                    fq = []
                    for mms, fin in qk_fill:
                        def qk_part(items):
                            def go():
                                for it in items:
                                    it()
                            return go
                        fq.append([qk_part(mms[0:4])])
                        fin_ = fin

                        def qk_rest(items=mms[4:8], f=fin_):
                            def go():
                                for it in items:
                                    it()
                                f()
                            return go
                        fq.append([qk_rest()])
                    # AV chains spread over mid/late m-steps, transposes after
                    av_sched = {3: av_items[0:2], 4: av_items[2:4],
                                5: av_items[4:6], 6: av_items[6:8]}
                    tr_sched = {5: tr_items[0:1], 6: tr_items[1:2],
                                7: tr_items[2:4]}

                    pts_now = []
                    for m in range(MT):
                        pts_now.append(s_pair(pr, n2, m))
                        if m < len(fq):
                            for it in fq[m]:
                                it()
                        for it in av_sched.get(m, []):
                            it()
                        for it in tr_sched.get(m, []):
                            it()
                        if m == MT - 1:
                            # leftover filler (slots with >8 filler groups)
                            for grp in fq[MT:]:
                                for it in grp:
                                    it()
                            if evac_item is not None:
                                evac_item()
                    pts_prev = (pr, n2, pts_now)

            # ---------------- tail ----------------
            # last slot's AV + transposes, then the rest of the projection
            ppr, pn2, ppts = pts_prev
            an_ts = [wp.tile([P, P], BF16, name=f"an{ppr}_{pn2}_{nu}",
                             tag="an", bufs=6) for nu in range(4)]
            tr_t = pp.tile([P, 512], BF16, name=f"tr{ppr}_{pn2}", tag="tr",
                           bufs=1)

            # fill the E(7,1,7) wait with two n2=0 proj chains on st halves
            st_tail = pp.tile([P, 1024], F32, name="st_tail", tag="st",
                              bufs=2)
            tail_fins = []
            for j, ot in enumerate((0, 1)):
                mms, fin = proj_chain_mms(0, ot,
                                          st_tail[:, j * 512:(j + 1) * 512])
                for go in mms[0:4]:
                    go()
                tail_fins.append((mms[4:], fin))

            for nu in range(4):
                for h in range(2):
                    av_chain(ppr, pn2, h, nu, ppts, an_ts[nu])
                if nu >= 1 and tail_fins:
                    mms, fin = tail_fins.pop(0)
                    for go in mms:
                        go()
                    fin()
            for mms, fin in tail_fins:
                for go in mms:
                    go()
                fin()
            for nu in range(4):
                nc.tensor.transpose(tr_t[:, nu * 128:(nu + 1) * 128],
                                    an_ts[nu][:], ident_t[:])
            nc.vector.tensor_copy(ao[ppr][:, pn2 * 512:(pn2 + 1) * 512],
                                  tr_t[:])

            # remaining projection: n2=0 ot=2..7, n2=1 ot=0..7
            remaining = [(0, ot) for ot in range(2, CT)]
            remaining += [(1, ot) for ot in range(CT)]
            tags = ["st2", "st2", "st3", "st3", "acc", "tr", "av", "av"]
            st2 = pp.tile([P, 1024], F32, name="st2", tag="st", bufs=2)
            st3 = pp.tile([P, 1024], F32, name="st3", tag="st", bufs=2)
            fins = []
            for i, (n2, ot) in enumerate(remaining):
                tg = tags[i % 8]
                if tg == "st2":
                    ps = st2[:, (i % 2) * 512:((i % 2) + 1) * 512]
                elif tg == "st3":
                    ps = st3[:, (i % 2) * 512:((i % 2) + 1) * 512]
                elif tg == "av":
                    ps = pp.tile([P, 512], F32, name=f"ytail{i}", tag="av",
                                 bufs=2)
                else:
                    ps = pp.tile([P, 512], F32, name=f"ytail{i}", tag=tg,
                                 bufs=1)
                last = (i == len(remaining) - 1)
                if not last:
                    mms, fin = proj_chain_mms(n2, ot, ps)
                    for go in mms:
                        go()
                    fins.append(fin)
                    # drain finishes with one-chain delay so bufs recycle
                    if len(fins) >= 2:
                        fins.pop(0)()
                else:
                    # split the last output tile into halves so its first
                    # bias-add/DMA overlaps the second half's matmuls
                    for half in range(2):
                        csl = slice(half * 256, (half + 1) * 256)
                        for pr in range(PAIRS):
                            nc.tensor.matmul(
                                ps[:, csl],
                                wpj[pr][:, ot * P:(ot + 1) * P],
                                ao[pr][:, n2 * 512 + half * 256:
                                       n2 * 512 + (half + 1) * 256],
                                start=(pr == 0), stop=(pr == PAIRS - 1),
                            )
                        yt = wp.tile([P, 256], BF16, name=f"ylast{half}",
                                     tag="y", bufs=3)
                        nc.vector.tensor_scalar_add(yt[:], ps[:, csl],
                                                    bias_t[:, ot:ot + 1])
                        nc.sync.dma_start(
                            outT_d.ap()[ot * P:(ot + 1) * P,
                                        n2 * 512 + half * 256:
                                        n2 * 512 + (half + 1) * 256],
                            yt[:])
                        if half == 0:
                            for fin in fins:
                                fin()
                            fins = []
            for fin in fins:
                fin()

    nc.compile()
    return nc


def get_nc():
    if "nc" not in _cache:
        _cache["nc"] = _build()
    return _cache["nc"]


def kernel(x, w_qkv, w_proj, b_proj):
    x = np.asarray(x, dtype=np.float32)
    w_qkv = np.asarray(w_qkv, dtype=np.float32)
    w_proj = np.asarray(w_proj, dtype=np.float32)
    b_proj = np.asarray(b_proj, dtype=np.float32)

    bf = ml_dtypes.bfloat16
    wqkvT = np.ascontiguousarray(w_qkv.T).astype(bf)     # [C, 3C]
    wqkT = np.ascontiguousarray(wqkvT[:, 0:2 * C])       # [C, 2C] q,k cols
    wprojT = np.ascontiguousarray(w_proj.T).astype(bf)   # [C, C]
    bias = np.ascontiguousarray(b_proj.reshape(CT, P).T).astype(np.float32)
    ident = np.eye(P, dtype=bf)

    in_maps = []
    wqv_host = wqkvT[:, 2 * C:]                          # [C, C] v columns
    for b in range(N_CORES):
        xT = np.ascontiguousarray(x[b].T).astype(bf)     # [C, N]
        xw = np.ascontiguousarray(np.concatenate([xT, wqv_host], axis=1))
        in_maps.append({"xw": xw, "wqkT": wqkT, "wprojT": wprojT,
                        "bias": bias, "ident": ident})

    nc = get_nc()
    _cache["in_maps"] = in_maps
    res = bass_utils.run_bass_kernel_spmd(nc, in_maps,
                                          core_ids=list(range(N_CORES)))
    out = np.empty((B, N, C), dtype=np.float32)
    for b in range(N_CORES):
        out[b] = res.results[b]["outT"].T.astype(np.float32)
    return out


# revision 19
# speedup vs baseline: 1.1069x; 1.0137x over previous
"""Trainium2 Bass kernel for batched multi-head self-attention.

Reference computation (per batch element b):
    qkv = x @ w_qkv.T                  # [N, 3C]
    q, k, v = split/reshape to heads   # H=16 heads, d=64
    attn = softmax(q @ k.T / sqrt(d))
    out = (attn @ v) reshaped back     # [N, C]
    y = out @ w_proj.T + b_proj

Sharding: pure data-parallel over batch B=8 across the 8 NeuronCores
(one batch element per core, weights replicated, no collectives).

On-device layout (everything transposed so matmuls contract over the
partition axis with no input transposes):
  - xT      [C, N]   (host pre-transposed, bf16)
  - wqkT    [C, 2C]  (host pre-transposed q,k columns, bf16)
  - wprojT  [C, C]   (host pre-transposed, bf16)

Cost model: a PE matmul costs out_free_size * 0.4167ns regardless of
contraction size or output partitions.  The key restructure vs the
229us baseline is the attn@V matmul orientation:
  - scores S^T [m,n] tiles (K=64, head pairs row-packed): free=512.
  - AV is computed FLIPPED: out[n, d+1] = pt[m, n-tile].T @ [V | 1]
    with free=65 instead of the old [d+1, 512] free=512 layout.
    This halves AV PE time (54.6us -> 27.7us).  The ones column gives
    softmax row sums per output partition, so normalization is a
    per-partition tensor_scalar divide on DVE (no partition
    broadcasts).
  - The [n, c] result is transposed back to [c, n] for the output
    projection with PE transpose instructions (128x128 via identity,
    53ns each, 64 total = 3.4us).
  - exp() is done on [128, 1024] head-pair tiles (one ACT instruction
    per (n2, m)) to amortize the ~185ns ACT access-latency overhead;
    ACT total 133us stays under the PE's ~197us.
Emission interleaves each attention slot (pr, n2) with the previous
slot's AV/transpose work and the next pair's q/k projection chains so
the PE never waits on ACT; the output projection fills the tail.
PSUM budget (8 banks): st 2x2 + acc 1 + av 2 + tr 1.
"""

import os
import sys

for _p in ("/opt/trn_rl_repo", "/root/.axon_site/_ro/trn_rl_repo"):
    if os.path.isdir(_p) and _p not in sys.path:
        sys.path.insert(0, _p)
        break

import numpy as np
import ml_dtypes

import concourse.bass as bass
import concourse.bacc as bacc
import concourse.tile as tile
import concourse.mybir as mybir
from concourse import bass_utils

BF16 = mybir.dt.bfloat16
F32 = mybir.dt.float32
AF = mybir.ActivationFunctionType
ALU = mybir.AluOpType

B, N, C, H = 8, 1024, 1024, 16
D = C // H            # 64 head dim
P = 128               # partitions
CT = C // P           # 8 contraction tiles
NT2 = N // 512        # 2 n-tiles of 512
MT = N // P           # 8 m-tiles of 128
PAIRS = H // 2        # 8 head pairs
SCALE = float(D) ** -0.5
N_CORES = 8

_cache = {}


def _build():
    nc = bacc.Bacc("TRN2", target_bir_lowering=False, debug=False,
                   enable_asserts=False, num_devices=N_CORES)

    xw_d = nc.dram_tensor("xw", [C, 2 * N], BF16, kind="ExternalInput")
    wqkT_d = nc.dram_tensor("wqkT", [C, 2 * C], BF16, kind="ExternalInput")
    wprojT_d = nc.dram_tensor("wprojT", [C, C], BF16, kind="ExternalInput")
    bias_d = nc.dram_tensor("bias", [P, CT], F32, kind="ExternalInput")
    ident_d = nc.dram_tensor("ident", [P, P], BF16, kind="ExternalInput")
    outT_d = nc.dram_tensor("outT", [C, N], BF16, kind="ExternalOutput")

    with tile.TileContext(nc) as tc:
        with (
            tc.tile_pool(name="res", bufs=1) as rp,
            tc.tile_pool(name="work", bufs=2) as wp,
            tc.tile_pool(name="ps", bufs=1, space="PSUM") as pp,
        ):
            # ---------------- PE warm-up ----------------
            # Cover the initial input-DMA wait with dummy matmuls so the
            # p-state/HAM ramp completes before real work arrives.
            warm_a = wp.tile([P, 512], BF16, name="warm_a", tag="warm_a",
                             bufs=1)
            nc.vector.memset(warm_a[:], 0.25)
            warm_ps = pp.tile([P, 512], F32, name="warm_ps", tag="acc",
                              bufs=1)
            for _ in range(6):
                nc.tensor.matmul(warm_ps[:], warm_a[:, 0:P], warm_a[:],
                                 start=True, stop=True)

            # ---------------- resident inputs ----------------
            xT = []
            wqv = []
            for i in range(CT):
                t = rp.tile([P, 2 * N], BF16, name=f"xw{i}", tag=f"xw{i}")
                nc.sync.dma_start(t[:], xw_d.ap()[i * P:(i + 1) * P, :])
                xT.append(t[:, 0:N])
                wqv.append(t[:, N:2 * N])
            wqk = []
            for i in range(CT):
                t = rp.tile([P, 2 * C], BF16, name=f"wqk{i}", tag=f"wqk{i}")
                nc.sync.dma_start(t[:], wqkT_d.ap()[i * P:(i + 1) * P, :])
                wqk.append(t)
            wpj = []
            for i in range(CT):
                t = rp.tile([P, C], BF16, name=f"wpj{i}", tag=f"wpj{i}")
                nc.sync.dma_start(t[:], wprojT_d.ap()[i * P:(i + 1) * P, :])
                wpj.append(t)
            bias_t = rp.tile([P, CT], F32, name="bias_t", tag="bias")
            nc.sync.dma_start(bias_t[:], bias_d.ap())
            ident_t = rp.tile([P, P], BF16, name="ident_t", tag="ident")
            nc.sync.dma_start(ident_t[:], ident_d.ap())

            # ---------------- result tiles ----------------
            qT = [rp.tile([P, N], BF16, name=f"qT{i}", tag=f"qT{i}")
                  for i in range(PAIRS)]
            kT = [rp.tile([P, N], BF16, name=f"kT{i}", tag=f"kT{i}")
                  for i in range(PAIRS)]
            # vt[m]: [128 m-rows, 16 heads, 64 v-dims + ones col]
            vt = [rp.tile([P, H, D + 1], BF16, name=f"vt{m}", tag=f"vt{m}")
                  for m in range(MT)]
            ao = [rp.tile([P, N], BF16, name=f"ao{i}", tag=f"ao{i}")
                  for i in range(PAIRS)]

            for m in range(MT):
                nc.vector.memset(vt[m][:, :, D:D + 1], 1.0)

            # ---------------- emission helpers ----------------
            def vproj_m(m):
                """v-projection for m-tile m: [128 tokens, 1024 v-dims]."""
                ps = pp.tile([P, 1024], F32, name=f"vps{m}", tag="st", bufs=2)
                for j in range(2):
                    for c in range(CT):
                        nc.tensor.matmul(
                            ps[:, j * 512:(j + 1) * 512],
                            xT[c][:, m * P:(m + 1) * P],
                            wqv[c][:, j * 512:(j + 1) * 512],
                            start=(c == 0), stop=(c == CT - 1),
                        )
                nc.vector.tensor_copy(
                    vt[m][:, :, 0:D],
                    ps[:].rearrange("p (h d) -> p h d", d=D),
                )

            def qk_chain_mms(pr, which, n2, tag):
                """Returns (list of mm closures, finish closure)."""
                o0 = which * C + pr * P
                nsl = slice(n2 * 512, (n2 + 1) * 512)
                ps = pp.tile([P, 512], F32, name=f"qk{pr}_{which}_{n2}",
                             tag=tag, bufs=1)
                dst = (qT if which == 0 else kT)[pr]

                def mk(c):
                    def go():
                        nc.tensor.matmul(
                            ps[:],
                            wqk[c][:, o0:o0 + P],
                            xT[c][:, nsl],
                            start=(c == 0), stop=(c == CT - 1),
                        )
                    return go

                def fin():
                    nc.vector.tensor_copy(dst[:, nsl], ps[:])

                return [mk(c) for c in range(CT)], fin

            def s_pair(pr, n2, m):
                """Score matmuls for both heads of the pair + combined exp.
                Returns the pt tile."""
                nsl = slice(n2 * 512, (n2 + 1) * 512)
                msl = slice(m * P, (m + 1) * P)
                st_t = pp.tile([P, 1024], F32, name=f"st{pr}_{n2}_{m}",
                               tag="st", bufs=2)
                for h in range(2):
                    psl = slice(h * 64, (h + 1) * 64)
                    nc.tensor.matmul(
                        st_t[:, h * 512:(h + 1) * 512],
                        kT[pr][psl, msl],
                        qT[pr][psl, nsl],
                        start=True, stop=True,
                        tile_position=(h * 64, 0),
                    )
                pt_t = wp.tile([P, 1024], BF16, name=f"pt{pr}_{n2}_{m}",
                               tag="pt", bufs=18)
                nc.scalar.activation(pt_t[:], st_t[:], AF.Exp, scale=SCALE)
                return pt_t

            def av_chain(pr, n2, h, nu, pts, an_t):
                """Flipped AV for one head and one 128-col n-tile."""
                head = 2 * pr + h
                av_t = pp.tile([P, D + 1], F32, name=f"av{pr}_{n2}_{h}_{nu}",
                               tag="av", bufs=2)
                lo = h * 512 + nu * 128
                for mi in range(MT):
                    nc.tensor.matmul(
                        av_t[:],
                        pts[mi][:, lo:lo + 128],
                        vt[mi][:, head, :],
                        start=(mi == 0), stop=(mi == MT - 1),
                    )
                # normalize + evacuate: per-partition scale by 1/sums col
                rec = wp.tile([P, 1], F32, name=f"rc{pr}_{n2}_{h}_{nu}",
                              tag="rec", bufs=4)
                nc.vector.reciprocal(rec[:], av_t[:, D:D + 1])
                nc.vector.tensor_scalar_mul(
                    an_t[:, h * 64:(h + 1) * 64], av_t[:, 0:D], rec[:])

            partials = {}    # (n2, ot) -> bf16 partial (pr 0..3 sum + bias)

            def proj_lo_chain(n2, ot):
                """First-half output projection (pr 0..3) with bias folded
                in, parked to SBUF; runs in the late ACT-gated slots."""
                ps = pp.tile([P, 512], F32, name=f"ylo{n2}_{ot}", tag="acc",
                             bufs=1)
                nsl = slice(n2 * 512, (n2 + 1) * 512)
                for pr in range(4):
                    nc.tensor.matmul(
                        ps[:],
                        wpj[pr][:, ot * P:(ot + 1) * P],
                        ao[pr][:, nsl],
                        start=(pr == 0), stop=(pr == 3),
                    )
                pt_ = wp.tile([P, 512], BF16, name=f"ypart{n2}_{ot}",
                              tag="part", bufs=6)
                nc.vector.tensor_scalar_add(pt_[:], ps[:],
                                            bias_t[:, ot:ot + 1])
                partials[(n2, ot)] = pt_

            def proj_hi_chain(n2, ot, ps):
                """Second half (pr 4..7) + partial add, for parked tiles."""
                nsl = slice(n2 * 512, (n2 + 1) * 512)
                for pr in range(4, PAIRS):
                    nc.tensor.matmul(
                        ps[:],
                        wpj[pr][:, ot * P:(ot + 1) * P],
                        ao[pr][:, nsl],
                        start=(pr == 4), stop=(pr == PAIRS - 1),
                    )

                def fin():
                    yt = wp.tile([P, 512], BF16, name=f"yh{ot}_{n2}",
                                 tag="y", bufs=3)
                    nc.vector.tensor_add(yt[:], ps[:],
                                         partials[(n2, ot)][:])
                    nc.sync.dma_start(outT_d.ap()[ot * P:(ot + 1) * P, nsl],
                                      yt[:])

                return fin

            def proj_chain_mms(n2, ot, ps):
                """Output projection chain closures for tile (n2, ot)."""
                nsl = slice(n2 * 512, (n2 + 1) * 512)

                def mk(pr):
                    def go():
                        nc.tensor.matmul(
                            ps[:],
                            wpj[pr][:, ot * P:(ot + 1) * P],
                            ao[pr][:, nsl],
                            start=(pr == 0), stop=(pr == PAIRS - 1),
                        )
                    return go

                def fin():
                    yt = wp.tile([P, 512], BF16, name=f"y{ot}_{n2}", tag="y",
                                 bufs=3)
                    nc.vector.tensor_scalar_add(yt[:], ps[:],
                                                bias_t[:, ot:ot + 1])
                    nc.sync.dma_start(outT_d.ap()[ot * P:(ot + 1) * P, nsl],
                                      yt[:])

                return [mk(pr) for pr in range(PAIRS)], fin

            # ---------------- startup ----------------
            # Phase 1: vproj m0..m3 c-OUTER with 8 simultaneously-open psum
            # chains (all 8 banks) so each arriving xw c-tile feeds 8 matmuls
            # immediately -- the PE tracks the DMA stream instead of stalling
            # for the full 4MB xw tensor.
            ps01 = [pp.tile([P, 1024], F32, name=f"vps{m}", tag="st", bufs=2)
                    for m in range(2)]
            ps23 = {(2, 0): pp.tile([P, 512], F32, name="vp2a", tag="acc",
                                    bufs=1),
                    (2, 1): pp.tile([P, 512], F32, name="vp2b", tag="tr",
                                    bufs=1),
                    (3, 0): pp.tile([P, 512], F32, name="vp3a", tag="av",
                                    bufs=2),
                    (3, 1): pp.tile([P, 512], F32, name="vp3b", tag="av",
                                    bufs=2)}
            for c in range(CT):
                for m in range(4):
                    for j in range(2):
                        dst = (ps01[m][:, j * 512:(j + 1) * 512] if m < 2
                               else ps23[(m, j)][:])
                        nc.tensor.matmul(
                            dst,
                            xT[c][:, m * P:(m + 1) * P],
                            wqv[c][:, j * 512:(j + 1) * 512],
                            start=(c == 0), stop=(c == CT - 1),
                        )
            for m in range(2):
                nc.vector.tensor_copy(
                    vt[m][:, :, 0:D],
                    ps01[m][:].rearrange("p (h d) -> p h d", d=D))
            for m in (2, 3):
                for j in range(2):
                    nc.vector.tensor_copy(
                        vt[m][:, j * 8:(j + 1) * 8, 0:D],
                        ps23[(m, j)][:].rearrange("p (h d) -> p h d", d=D))

            # Phase 2: vproj m4..m7 as m-chains, then the pair-0 qk chains
            # (their wqk DMAs land only after all of xw).
            for m in range(4, MT):
                vproj_m(m)
            for which, n2, tag in ((0, 0, "acc"), (0, 1, "tr"),
                                   (1, 0, "acc"), (1, 1, "tr")):
                mms, fin = qk_chain_mms(0, which, n2, tag)
                for go in mms:
                    go()
                fin()

            # ---------------- main loop ----------------
            # Slot (pr, n2).  Filler work per slot:
            #  - previous slot's AV chains + divides + transposes + ao evac
            #  - next pair's q (n2=0 slot) / k (n2=1 slot) projection chains
            #  - vproj m6/m7 in slot (0,0); output projection at pr=7
            pts_prev = None      # (pr, n2, [pt tiles]) of previous slot

            for pr in range(PAIRS):
                for n2 in range(NT2):
                    slot = 2 * pr + n2
                    # --- gather filler: qk chains of next pair ---
                    qk_fill = []
                    if pr < PAIRS - 1:
                        which = n2          # q chains in n2=0, k in n2=1
                        qk_fill.append(
                            qk_chain_mms(pr + 1, which, 0, "acc"))
                        qk_fill.append(
                            qk_chain_mms(pr + 1, which, 1, "tr"))

                    # --- previous slot's AV work ---
                    if pts_prev is not None:
                        ppr, pn2, ppts = pts_prev
                        an_ts = [wp.tile([P, P], BF16,
                                         name=f"an{ppr}_{pn2}_{nu}",
                                         tag="an", bufs=6)
                                 for nu in range(4)]
                        tr_t = pp.tile([P, 512], BF16,
                                       name=f"tr{ppr}_{pn2}", tag="tr",
                                       bufs=1)

                        def mk_av(nu, h, _ppr=ppr, _pn2=pn2, _ppts=ppts,
                                  _an=an_ts):
                            def go():
                                av_chain(_ppr, _pn2, h, nu, _ppts, _an[nu])
                            return go

                        def mk_tr(nu, _an=an_ts, _tr=tr_t):
                            def go():
                                nc.tensor.transpose(
                                    _tr[:, nu * 128:(nu + 1) * 128],
                                    _an[nu][:], ident_t[:])
                            return go

                        def mk_evac(_ppr=ppr, _pn2=pn2, _tr=tr_t):
                            def go():
                                nc.vector.tensor_copy(
                                    ao[_ppr][:, _pn2 * 512:(_pn2 + 1) * 512],
                                    _tr[:])
                            return go

                        av_items = [mk_av(nu, h)
                                    for nu in range(4) for h in range(2)]
                        tr_items = [mk_tr(nu) for nu in range(4)]
                        evac_item = mk_evac()
                    else:
                        av_items, tr_items, evac_item = [], [], None

                    # --- weave the slot ---
                    # filler queue: list of closure-lists, consumed in order
                    # across the 8 m-steps.
                    fq = []
                    if pr == PAIRS - 1:
                        # ACT-gated last slots: fill with partial outproj
                        los = ([(0, 2), (0, 3), (0, 4)] if n2 == 0
                               else [(0, 5), (1, 0), (1, 1)])
                        for lo_n2, lo_ot in los:
                            fq.append([lambda a=lo_n2, b=lo_ot:
                                       proj_lo_chain(a, b)])
                    for mms, fin in qk_fill:
                        def qk_part(items):
                            def go():
                                for it in items:
                                    it()
                            return go
                        fq.append([qk_part(mms[0:4])])
                        fin_ = fin

                        def qk_rest(items=mms[4:8], f=fin_):
                            def go():
                                for it in items:
                                    it()
                                f()
                            return go
                        fq.append([qk_rest()])
                    # AV chains spread over mid/late m-steps, transposes after
                    av_sched = {3: av_items[0:2], 4: av_items[2:4],
                                5: av_items[4:6], 6: av_items[6:8]}
                    tr_sched = {5: tr_items[0:1], 6: tr_items[1:2],
                                7: tr_items[2:4]}

                    pts_now = []
                    for m in range(MT):
                        pts_now.append(s_pair(pr, n2, m))
                        if m < len(fq):
                            for it in fq[m]:
                                it()
                        for it in av_sched.get(m, []):
                            it()
                        for it in tr_sched.get(m, []):
                            it()
                        if m == MT - 1:
                            # leftover filler (slots with >8 filler groups)
                            for grp in fq[MT:]:
                                for it in grp:
                                    it()
                            if evac_item is not None:
                                evac_item()
                    pts_prev = (pr, n2, pts_now)

            # ---------------- tail ----------------
            # last slot's AV + transposes, then the rest of the projection
            ppr, pn2, ppts = pts_prev
            an_ts = [wp.tile([P, P], BF16, name=f"an{ppr}_{pn2}_{nu}",
                             tag="an", bufs=6) for nu in range(4)]
            tr_t = pp.tile([P, 512], BF16, name=f"tr{ppr}_{pn2}", tag="tr",
                           bufs=1)

            # fill the E(7,1,7) wait with two n2=0 proj chains on st halves
            st_tail = pp.tile([P, 1024], F32, name="st_tail", tag="st",
                              bufs=2)
            tail_fins = []
            for j, ot in enumerate((0, 1)):
                mms, fin = proj_chain_mms(0, ot,
                                          st_tail[:, j * 512:(j + 1) * 512])
                for go in mms[0:4]:
                    go()
                tail_fins.append((mms[4:], fin))

            for nu in range(4):
                for h in range(2):
                    av_chain(ppr, pn2, h, nu, ppts, an_ts[nu])
                if nu >= 1 and tail_fins:
                    mms, fin = tail_fins.pop(0)
                    for go in mms:
                        go()
                    fin()
            for mms, fin in tail_fins:
                for go in mms:
                    go()
                fin()
            for nu in range(4):
                nc.tensor.transpose(tr_t[:, nu * 128:(nu + 1) * 128],
                                    an_ts[nu][:], ident_t[:])
            nc.vector.tensor_copy(ao[ppr][:, pn2 * 512:(pn2 + 1) * 512],
                                  tr_t[:])

            # remaining projection: hi-halves for the parked tiles first,
            # then the full chains; n2=1 ot=7 (split) stays last.
            remaining = [(0, ot) for ot in range(2, CT)]
            remaining += [(1, ot) for ot in range(CT)]
            remaining.sort(key=lambda t: (t not in partials, t))
            tags = ["st2", "st2", "st3", "st3", "acc", "tr", "av", "av"]
            st2 = pp.tile([P, 1024], F32, name="st2", tag="st", bufs=2)
            st3 = pp.tile([P, 1024], F32, name="st3", tag="st", bufs=2)
            fins = []
            for i, (n2, ot) in enumerate(remaining):
                tg = tags[i % 8]
                if tg == "st2":
                    ps = st2[:, (i % 2) * 512:((i % 2) + 1) * 512]
                elif tg == "st3":
                    ps = st3[:, (i % 2) * 512:((i % 2) + 1) * 512]
                elif tg == "av":
                    ps = pp.tile([P, 512], F32, name=f"ytail{i}", tag="av",
                                 bufs=2)
                else:
                    ps = pp.tile([P, 512], F32, name=f"ytail{i}", tag=tg,
                                 bufs=1)
                last = (i == len(remaining) - 1)
                if (n2, ot) in partials:
                    fins.append(proj_hi_chain(n2, ot, ps))
                    if len(fins) >= 2:
                        fins.pop(0)()
                elif not last:
                    mms, fin = proj_chain_mms(n2, ot, ps)
                    for go in mms:
                        go()
                    fins.append(fin)
                    # drain finishes with one-chain delay so bufs recycle
                    if len(fins) >= 2:
                        fins.pop(0)()
                else:
                    # split the last output tile into halves so its first
                    # bias-add/DMA overlaps the second half's matmuls
                    for half in range(2):
                        csl = slice(half * 256, (half + 1) * 256)
                        for pr in range(PAIRS):
                            nc.tensor.matmul(
                                ps[:, csl],
                                wpj[pr][:, ot * P:(ot + 1) * P],
                                ao[pr][:, n2 * 512 + half * 256:
                                       n2 * 512 + (half + 1) * 256],
                                start=(pr == 0), stop=(pr == PAIRS - 1),
                            )
                        yt = wp.tile([P, 256], BF16, name=f"ylast{half}",
                                     tag="y", bufs=3)
                        nc.vector.tensor_scalar_add(yt[:], ps[:, csl],
                                                    bias_t[:, ot:ot + 1])
                        nc.sync.dma_start(
                            outT_d.ap()[ot * P:(ot + 1) * P,
                                        n2 * 512 + half * 256:
                                        n2 * 512 + (half + 1) * 256],
                            yt[:])
                        if half == 0:
                            for fin in fins:
                                fin()
                            fins = []
            for fin in fins:
                fin()

    nc.compile()
    return nc


def get_nc():
    if "nc" not in _cache:
        _cache["nc"] = _build()
    return _cache["nc"]


def kernel(x, w_qkv, w_proj, b_proj):
    x = np.asarray(x, dtype=np.float32)
    w_qkv = np.asarray(w_qkv, dtype=np.float32)
    w_proj = np.asarray(w_proj, dtype=np.float32)
    b_proj = np.asarray(b_proj, dtype=np.float32)

    bf = ml_dtypes.bfloat16
    wqkvT = np.ascontiguousarray(w_qkv.T).astype(bf)     # [C, 3C]
    wqkT = np.ascontiguousarray(wqkvT[:, 0:2 * C])       # [C, 2C] q,k cols
    wprojT = np.ascontiguousarray(w_proj.T).astype(bf)   # [C, C]
    bias = np.ascontiguousarray(b_proj.reshape(CT, P).T).astype(np.float32)
    ident = np.eye(P, dtype=bf)

    in_maps = []
    wqv_host = wqkvT[:, 2 * C:]                          # [C, C] v columns
    for b in range(N_CORES):
        xT = np.ascontiguousarray(x[b].T).astype(bf)     # [C, N]
        xw = np.ascontiguousarray(np.concatenate([xT, wqv_host], axis=1))
        in_maps.append({"xw": xw, "wqkT": wqkT, "wprojT": wprojT,
                        "bias": bias, "ident": ident})

    nc = get_nc()
    _cache["in_maps"] = in_maps
    res = bass_utils.run_bass_kernel_spmd(nc, in_maps,
                                          core_ids=list(range(N_CORES)))
    out = np.empty((B, N, C), dtype=np.float32)
    for b in range(N_CORES):
        out[b] = res.results[b]["outT"].T.astype(np.float32)
    return out
